# revision 1
# baseline (speedup 1.0000x reference)
"""TRN2 Bass kernel for nn_BatchGraphEncoder (gnn_message_passing).

Strategy
--------
Data-parallel over batch B=8 (one scene per NeuronCore). The A x A edge state
is collapsed algebraically: es_new[i,j] = h[i]@W1 + h[j]@W2 + es[i,j]@W3 + b,
and the only consumption of es is its row-sum agg[i] = sum_j es_new[i,j], so
we track S[i] = sum_j es[i,j] with the recurrence
    S_new = A*(h@W1) + (sum_j h[j])@W2 + S@W3 + A*b_edge.
This removes all O(A^2) work. All layouts are feature-major [feat, col] with
column index c = t*64 + a ("t-major").

Precision: the recurrence is chaotic (perturbation gain ~2x/step), so fp16
storage is not enough, and fp32 matmuls on TRN2 are ~12x slower than fp16.
We use an fp16 hi/lo split (x = xh + xl) with 3-term products
    x@W ~= xh@Wh + xh@Wl + xl@Wh        (error ~2^-22, fp32-like)
running everything on fast fp16 matmuls with fp32 PSUM accumulation.

The LSTM gate nonlinearities use a tanh-only trick: sigmoid(x)=0.5tanh(0.5x)
+0.5, with the i/f/o weight columns pre-scaled by 0.5 on the host so one ACT
tanh covers all four gates; h is tracked as h~ = 2h with all h-consuming
weights pre-halved.
"""
import numpy as np

import bass_rust
import concourse.bass as bass
import concourse.tile as tile
from concourse import mybir
from concourse.bass_utils import run_bass_kernel_spmd
from concourse.vector_clock import ScopedClock, VectorClock

B, A, T, H, E, O, TY, SC, AG = 8, 64, 12, 128, 128, 128, 8, 32, 16
NCOL = T * A           # 768 columns, c = t*64 + a
F16 = mybir.dt.float16
F32 = mybir.dt.float32
AF = mybir.ActivationFunctionType
ALU = mybir.AluOpType

# ---------------------------------------------------------------------------
# walrus rejects instructions with >4 sync-wait commands; TileContext's exit
# drain collects a wait for every live semaphore onto one Drain. Split them.
_N_PROCS = bass_rust.N_PROCS


def _patched_drain_and_barrier(self, tick_clock, wait_clock):
    gc = tick_clock.global_clock
    ticks = [gc.peek_next(p) - 1 for p in range(_N_PROCS)]
    nz = [p for p, t in enumerate(ticks) if t > 0]
    for i in range(0, len(nz), 1):
        chunk = set(nz[i:i + 1])
        part = VectorClock([ticks[p] if p in chunk else 0 for p in range(_N_PROCS)])
        d = self.nc.sync.drain()
        wait_clock.add_sem_waits(d.ins, ScopedClock({None: part}))
    self.nc.sync.drain()
    # single-shot NEFF: skip the exit barriers and semaphore clears (they cost
    # ~6us of EVSEM butterflies); each engine halts after its last instruction.
    assert self.sems is not None
    popped = self.nc._tile_sem_poison_stack.pop()
    assert popped is self._sem_poison


tile.TileContext._drain_and_barrier = _patched_drain_and_barrier


def _split_excess_waits(nc, limit=1):
    """walrus accepts only ~1 sync-wait command per TPB instruction; Tile can
    assign several. Move excess waits onto ENGINE_NOPs inserted just before
    the over-subscribed instruction (same engine => program order preserves
    the happens-before)."""
    eligible = {mybir.EngineType.PE, mybir.EngineType.DVE,
                mybir.EngineType.Activation, mybir.EngineType.Pool,
                mybir.EngineType.SP}
    n_split = 0
    for f in nc.m.functions:
        for bb in f.blocks:
            insts = bb.instructions
            i = 0
            while i < len(insts):
                inst = insts[i]
                si = inst.sync_info
                if (si is not None and len(si.on_wait) > limit
                        and inst.engine in eligible):
                    waits = list(si.on_wait)
                    extra, keep = waits[:-limit], waits[-limit:]
                    pos = i
                    for j in range(0, len(extra), limit):
                        chunk = extra[j:j + limit]
                        ev = mybir.InstEventSemaphore(
                            name=nc.get_next_instruction_name(), ins=[], outs=[])
                        ev.engine = inst.engine
                        ev.sync_info = bass_rust.SyncInfo(on_wait=chunk, on_update=[])
                        nc.register_instruction(ev)
                        insts.insert(pos, ev)
                        pos += 1
                        i += 1
                        n_split += 1
                    si.on_wait = keep
                i += 1
    return n_split


# ---------------------------------------------------------------------------
# host-side weight composition
def _s16(x):
    """fp16 hi/lo split of an fp64/fp32 array."""
    x = np.asarray(x, np.float64)
    h = x.astype(np.float16)
    l = (x - h.astype(np.float64)).astype(np.float16)
    return h, l


class _Packer:
    """Packs [K<=128, M] fp16 blocks into one [128, total] blob."""

    def __init__(self):
        self.cols = 0
        self.items = {}
        self.blocks = []

    def add(self, name, arr):
        arr = np.asarray(arr, np.float16)
        assert arr.ndim == 2 and arr.shape[0] <= 128, (name, arr.shape)
        self.items[name] = (self.cols, arr.shape[0], arr.shape[1])
        self.blocks.append(arr)
        self.cols += arr.shape[1]

    def blob(self):
        out = np.zeros((128, self.cols), np.float16)
        for (name, (off, k, m)), a in zip(self.items.items(), self.blocks):
            out[:k, off:off + m] = a
        return out


def _prep_weights(inp):
    """Compose all lhsT weight tiles (fp64 math, fp16 hi/lo) + fp32 vectors."""
    W_ih = np.asarray(inp["W_ih"], np.float64)
    W_hh = np.asarray(inp["W_hh"], np.float64)
    b_g = np.asarray(inp["b_ih"], np.float64) + np.asarray(inp["b_hh"], np.float64)
    # gate order (f,i,g,o): f's ACT output is sigmoid(zf) directly (so the
    # c update is a pure tensor-tensor multiply, legal on GPSIMD); i/o keep
    # the 0.5 tanh trick; g unscaled
    perm = np.concatenate([np.arange(H, 2 * H), np.arange(0, H),
                           np.arange(2 * H, 3 * H), np.arange(3 * H, 4 * H)])
    W_ih = W_ih[:, perm]
    W_hh = W_hh[:, perm]
    b_g = b_g[perm]
    cs = np.concatenate([np.full(2 * H, 0.5), np.ones(H), np.full(H, 0.5)])
    W_ih = W_ih * cs
    W_hh = W_hh * cs
    b_g = b_g * cs

    Wih_c = W_ih[0:H]
    Wih_ty = W_ih[H:2 * H]
    Wih_ne = W_ih[2 * H:3 * H]
    Wih_sc = W_ih[3 * H:4 * H]
    Wih_ag = W_ih[4 * H:5 * H]

    We = np.asarray(inp["W_edge"], np.float64)
    W1, W2, W3 = We[:H], We[H:2 * H], We[2 * H:]
    Ws = np.asarray(inp["W_self"], np.float64)
    Wsh, Wss, Wst = Ws[:H], Ws[H:H + E], Ws[H + E:]
    We2n = np.asarray(inp["W_e2n"], np.float64)
    Wnt = np.asarray(inp["W_node_type"], np.float64)
    Wsc = np.asarray(inp["W_scene"], np.float64)
    Wag = np.asarray(inp["W_agent"], np.float64)
    b_edge = np.asarray(inp["b_edge"], np.float64)
    b_self = np.asarray(inp["b_self"], np.float64)
    b_e2n = np.asarray(inp["b_e2n"], np.float64)

    Pe = _Packer()          # needed for coord/G_pre (first compute phase)
    Pl = _Packer()          # recurrence/pred weights (needed ~10us later)

    def add_hl(name, arr, P=None):
        P = P if P is not None else Pl
        h, l = _s16(arr)
        P.add(name + "h", h)
        P.add(name + "l", l)

    # coord = relu(X6@W_in + b_in)
    W_in = np.asarray(inp["W_in"], np.float64)
    Winh, Winl = _s16(W_in)
    Pe.add("Winh", Winh)                          # K=6
    Pe.add("Winl", Winl)

    # G_pre coord chunk (K=128, 3-term)
    add_hl("Wcg", Wih_c, Pe)                          # [128,512] x2

    # G_pre merged chunk rows: [agent(16); ty(8); sc(32); rel(1); ones(1)]
    # (agent first: DVE writes need quadrant-aligned partition starts).
    # The ones row carries the full gate bias so the G windows need no
    # separate bias add (G32 written via a plain ACT Copy).
    ne_cst = A * b_edge @ We2n + b_self @ We2n + b_e2n
    bg_full = (b_g + np.asarray(inp["b_node_type"], np.float64) @ Wih_ty
               + np.asarray(inp["b_scene"], np.float64) @ Wih_sc + ne_cst @ Wih_ne)
    Wmg = np.vstack([Wag @ Wih_ag,
                     Wnt @ Wih_ty + Wst @ We2n @ Wih_ne,
                     Wsc @ Wih_sc,
                     (np.asarray(inp["b_agent"], np.float64) @ Wih_ag)[None],
                     bg_full[None]])
    Wmgh_, Wmgl_ = _s16(Wmg)                      # [58,512]
    Pe.add("WmgA", np.vstack([Wmgh_, np.zeros((6, 512), np.float16), Wmgh_]))
    Pe.add("Wmgl", Wmgl_)

    # t=0 correction: remove ne-coupled ty composite + ne const biases
    Cty = -(Wst @ We2n @ Wih_ne)
    Db0 = -(ne_cst @ Wih_ne)
    Ctyh, Ctyl = _s16(Cty)
    Dbh, Dbl = _s16(Db0[None])
    Pe.add("Wcorr", np.vstack([Ctyh, Ctyl, Ctyh, Dbh, Dbl]))  # rhs corrstk K=33

    # recurrence composites (h~=2h folding: h-consumers x0.5)
    add_hl("W3e", W3 @ We2n)
    add_hl("Wsse", Wss @ We2n)
    add_hl("Wcoh", ((A * W1 + Wsh) * 0.5) @ We2n)
    add_hl("W2eh", (W2 * 0.5) @ We2n)
    add_hl("W3", W3)
    add_hl("AW1h", A * W1 * 0.5)
    add_hl("W2h", W2 * 0.5)
    add_hl("Wss", Wss)
    add_hl("Wsh2", Wsh * 0.5)
    add_hl("Wihne", Wih_ne)                       # [128,512] x2
    add_hl("Whh2", W_hh * 0.5)                    # [128,512] x2
    add_hl("Wpredh", np.asarray(inp["W_pred"], np.float64) * 0.5)
    Wsth, Wstl = _s16(Wst)
    Pl.add("Wstk", np.vstack([Wsth, Wstl, Wsth]))  # rhs tystk K=24

    # S init: S_1 = f0ty@(A*Wp@W3) + 1*(sum_j f0ty_j)@(Wq@W3) + const
    Wei = np.asarray(inp["W_edge_in"], np.float64)
    Wet = np.asarray(inp["W_edge_type"], np.float64)
    Wp = np.vstack([Wei[0:6], Wei[12:20] + Wet[0:8]])
    Wq = np.vstack([Wei[6:12], Wei[20:28] + Wet[8:16]])
    Wp3 = A * (Wp @ W3)
    Wq3 = Wq @ W3

    def stk6(w):                                  # [14,128] -> rhs s1stk K=42
        wh, wl = _s16(w)
        return np.vstack([wh[0:6], wl[0:6], wh[0:6],
                          wh[6:14], wl[6:14], wh[6:14]])

    Pl.add("Wp3s", stk6(Wp3))
    Pl.add("Wq3s", stk6(Wq3))
    Pl.add("I16", np.eye(128, dtype=np.float16))

    cst = (np.asarray(inp["b_edge_in"], np.float64)
           + np.asarray(inp["b_edge_type"], np.float64))
    vec32 = np.zeros((128, 9), np.float32)
    vec32[:, 0] = np.asarray(inp["b_in"], np.float64)
    for g in range(4):
        vec32[:, 1 + g] = bg_full[g * H:(g + 1) * H]
    vec32[:, 5] = A * b_edge
    vec32[:, 6] = b_self
    vec32[:, 7] = A * (cst @ W3) + A * b_edge
    vec32[:, 8] = np.asarray(inp["b_pred"], np.float64)
    return Pe, Pl, vec32


_VN = {"b_in": 0, "bg0": 1, "bg1": 2, "bg2": 3, "bg3": 4,
       "sbias": 5, "bself": 6, "s1c": 7, "b_pred": 8}


def _prep_core_inputs(inp, b):
    """Per-core marshaled inputs packed into ONE fp16 blob [128, 2432].
    Column ranges (quadrant-stacked to keep DMA count minimal):
      0:768     Xch@p0, Xagh@p32, Xcl@p64, Xagl@p96
      768:1536  msth@p0, mstl@p64
      1536:2304 rel16@p0 (16 rows)
      2304:2368 s1s@p0 (42), tys@p64 (24)
      2368:2432 corrs@p0 (26)
    """
    norm = np.asarray(inp["normalized_trajectories"][b], np.float64)
    traj = np.asarray(inp["trajectories"][b], np.float64)
    ag = np.asarray(inp["agent_data"][b], np.float64)
    ty = np.asarray(inp["agent_type"][b], np.float64)        # [A,TY]
    sc = np.asarray(inp["scene_data"][b], np.float64)        # [T,SC]
    rel = np.asarray(inp["relevant_agents"][b], np.float64)  # [A]

    Xc = np.concatenate([norm, traj], -1).transpose(2, 1, 0).reshape(6, NCOL)
    Xag = ag.transpose(2, 1, 0).reshape(16, NCOL)
    Xch, Xcl = _s16(Xc)
    Xagh, Xagl = _s16(Xag)
    tyh, tyl = _s16(ty.T)                                    # [8,A]
    sch, scl = _s16(sc.T)                                    # [32,T]

    tyb_h = np.repeat(tyh[:, None, :], T, 1).reshape(8, NCOL)
    tyb_l = np.repeat(tyl[:, None, :], T, 1).reshape(8, NCOL)
    scb_h = np.repeat(sch[:, :, None], A, 2).reshape(32, NCOL)
    scb_l = np.repeat(scl[:, :, None], A, 2).reshape(32, NCOL)
    relrow = np.tile(rel, T)[None]                           # [1,768]
    relh, _ = _s16(relrow)
    z16 = np.zeros((16, NCOL), np.float16)
    z1 = np.zeros((1, NCOL), np.float16)
    one1 = np.ones((1, NCOL), np.float16)
    msth0 = np.vstack([z16, tyb_h, scb_h, relh, one1]).astype(np.float16)  # [58,768]
    mstl0 = np.vstack([z16, tyb_l, scb_l, z1, z1]).astype(np.float16)

    ones = np.ones((1, A), np.float16)
    corrstk = np.vstack([tyh, tyh, tyl, ones, ones])         # [26,A]
    tystk = np.vstack([tyh, tyh, tyl])                       # [24,A]
    f0h, f0l = Xch[0:6, 0:A], Xcl[0:6, 0:A]
    s1stk = np.vstack([f0h, f0h, f0l, tyh, tyh, tyl])        # [42,A]

    ones_fast = bool(np.all(rel == 1.0))
    if ones_fast:
        # mask multiply is an exact identity: agent rows go straight in
        msth0[0:16] = Xagh
        mstl0[0:16] = Xagl
        blob = np.zeros((128, 3264), np.float16)
        blob[0:6, 0:768] = Xch
        blob[0:6, 768:1536] = Xcl
        blob[0:58, 1536:2304] = msth0
        blob[64:122, 1536:2304] = mstl0
        blob[0:42, 3072:3136] = s1stk
        blob[0:24, 3136:3200] = tystk
        blob[0:26, 3200:3264] = corrstk
    else:
        blob = np.zeros((128, 5568), np.float16)
        blob[0:6, 0:768] = Xch
        blob[0:6, 768:1536] = Xcl
        blob[0:58, 1536:2304] = msth0
        blob[64:122, 1536:2304] = mstl0
        blob[0:42, 3072:3136] = s1stk
        blob[0:24, 3136:3200] = tystk
        blob[0:26, 3200:3264] = corrstk
        blob[0:16, 3264:4032] = Xagh
        blob[0:16, 4032:4800] = Xagl
        blob[0:16, 4800:5568] = np.repeat(relrow, 16, 0).astype(np.float16)
    return {"inblob": blob, "ones_fast": ones_fast}


# ---------------------------------------------------------------------------
def _build(nc, emap, ecols, lmap, lcols, ones_fast=True):
    """Emit the single-core program. emap/lmap: name -> (off, k, m)."""
    icols = 3264 if ones_fast else 5568
    inb_ap = nc.dram_tensor("inblob", [128, icols], F16, kind="ExternalInput").ap()
    vec_ap = nc.dram_tensor("vec32", [128, 9], F32, kind="ExternalInput").ap()
    id_ap = nc.dram_tensor("ident32", [128, 128], F32, kind="ExternalInput").ap()
    we_ap = nc.dram_tensor("wearly", [128, ecols], F16, kind="ExternalInput").ap()
    wl_ap = nc.dram_tensor("wlate", [128, lcols], F16, kind="ExternalInput").ap()
    out_ap = nc.dram_tensor("out", [128, NCOL], F32, kind="ExternalOutput").ap()

    dbg = {}

    with tile.TileContext(nc) as tc:
        with (
            tc.tile_pool(name="per", bufs=1) as per,
            tc.tile_pool(name="stp", bufs=3) as stp,
            tc.tile_pool(name="psG", bufs=2, space="PSUM") as psG,
            tc.tile_pool(name="psg", bufs=2, space="PSUM") as psg,
            tc.tile_pool(name="psn", bufs=2, space="PSUM") as psn,
            tc.tile_pool(name="pss", bufs=2, space="PSUM") as pss,
        ):
            # ---- persistent tiles + DMAs (small first, then early weights,
            # then late weights, so compute can start ASAP)
            we = per.tile([128, ecols], F16)
            wl = per.tile([128, lcols], F16)

            def W(name):
                if name in emap:
                    off, k, m = emap[name]
                    return we[0:k, off:off + m]
                off, k, m = lmap[name]
                return wl[0:k, off:off + m]

            def Wg(name, g):
                if name in emap:
                    off, k, m = emap[name]
                    return we[0:k, off + g * 128:off + (g + 1) * 128]
                off, k, m = lmap[name]
                return wl[0:k, off + g * 128:off + (g + 1) * 128]

            vec = per.tile([128, 9], F32)

            def V(name):
                i = _VN[name]
                return vec[:, i:i + 1]

            inb = per.tile([128, icols], F16)
            i32 = per.tile([128, 128], F32)
            nc.sync.dma_start(inb[:], inb_ap)
            nc.sync.dma_start(vec[:], vec_ap)
            nc.sync.dma_start(i32[:], id_ap)
            nc.sync.dma_start(we[:], we_ap)
            nc.sync.dma_start(wl[:], wl_ap)
            Xch = inb[0:6, 0:768]
            Xcl = inb[0:6, 768:1536]
            mstk = inb[0:122, 1536:2304]
            msth = inb[0:58, 1536:2304]
            mstl = inb[64:122, 1536:2304]
            s1s = inb[0:42, 3072:3136]
            tys = inb[0:24, 3136:3200]
            corrs = inb[0:26, 3200:3264]

            G32 = per.tile([128, 4 * NCOL], F32)     # [128, g*768 + t*64 + a]
            coh = per.tile([128, NCOL], F16)
            col = per.tile([128, NCOL], F16)
            hah = per.tile([128, NCOL], F16)         # h~ hi, col t*64+a
            hal = per.tile([128, NCOL], F16)
            out32 = per.tile([128, NCOL], F32)
            dbg.update(G32=G32, coh=coh, col=col, hah=hah, hal=hal)

            # warm the ACT tanh/relu table set during the DMAs
            warm = per.tile([1, 1], F32)
            nc.vector.memset(warm[:], 0.0)
            warm2 = per.tile([1, 1], F32)
            nc.scalar.activation(warm2[:], warm[:], AF.Tanh)

            nc.vector.memset(hal[:, 9 * 64:NCOL], 0.0)

            # spin the PE on a zeroed scratch tile during the ~8us DMA wait so
            # the HAM clock-gate reaches 8/8 before the real matmuls start
            scr = per.tile([128, 512], F16)
            nc.vector.memset(scr[:], 0.0)
            for k in range(14):
                wps = psn.tile([128, 512], F32, name=f"warm{k}", tag="n")
                nc.tensor.matmul(wps[:], scr[0:128, 0:128], scr[:],
                                 start=True, stop=True)

            # spin the PE on the first-arriving small tensor for ~3.5us so the
            # HAM clock-gate reaches 8/8 before the real matmuls start


            if not ones_fast:
                # mask agent rows: m = (Xagh+Xagl) * rel (fp32), hi/lo into mst
                Xagh = inb[0:16, 3264:4032]
                Xagl = inb[0:16, 4032:4800]
                rel16 = inb[0:16, 4800:5568]
                ms32 = per.tile([16, NCOL], F32)
                nc.vector.tensor_add(ms32[:], Xagh, Xagl)
                m32 = per.tile([16, NCOL], F32)
                nc.vector.tensor_mul(m32[:], ms32[:], rel16)
                nc.vector.tensor_copy(msth[0:16, :], m32[:])
                nc.vector.tensor_sub(mstl[0:16, :], m32[:], msth[0:16, :])

            # ---- coord = relu(X6@W_in + b_in), hi/lo
            co32 = per.tile([128, NCOL], F32)
            for hf in range(2):
                s = slice(hf * 384, hf * 384 + 384)
                cps = psG.tile([128, 384], F32, name=f"cops{hf}", tag="G")
                nc.tensor.matmul(cps[:], W("Winh"), Xch[:, s], start=True, stop=False)
                nc.tensor.matmul(cps[:], W("Winl"), Xch[:, s], start=False, stop=False)
                nc.tensor.matmul(cps[:], W("Winh"), Xcl[:, s], start=False, stop=True)
                nc.scalar.activation(co32[:, s], cps[:], AF.Relu, bias=V("b_in"))
            nc.vector.tensor_copy(coh[:], co32[:])
            nc.vector.tensor_sub(col[:], co32[:], coh[:])

            # ---- G_pre: per (g, col-window): coord 3-term + merged 2 MMs
            def emit_G2(g, c0, c1):
                    s = slice(c0, c1)
                    gps = psG.tile([128, c1 - c0], F32, name=f"gps{g}_{c0}", tag="G")
                    nc.tensor.matmul(gps[:], Wg("Wcgh", g), coh[:, s], start=True, stop=False)
                    nc.tensor.matmul(gps[:], Wg("Wcgl", g), coh[:, s], start=False, stop=False)
                    nc.tensor.matmul(gps[:], Wg("Wcgh", g), col[:, s], start=False, stop=False)
                    nc.tensor.matmul(gps[:], Wg("WmgA", g), mstk[:, s], start=False, stop=False)
                    nc.tensor.matmul(gps[:], Wg("Wmgl", g), msth[:, s], start=False, stop=True)
                    if c0 == 0:
                        # t=0 correction accumulates onto the closed group
                        nc.tensor.matmul(gps[:, 0:64], Wg("Wcorr", g), corrs[:],
                                         start=False, stop=True, skip_group_check=True)
                    o = g * NCOL + c0
                    nc.scalar.activation(G32[:, o:o + (c1 - c0)], gps[:], AF.Copy)

            for g in range(4):
                emit_G2(g, 0, 64)       # just the t=0 slice: unblocks t0 LSTM

            # ---- S1 / se1 init
            sps = pss.tile([128, 128], F32, name="sps0", tag="s")
            nc.tensor.matmul(sps[:, 0:64], W("Wp3s"), s1s[:], start=True, stop=True)
            qps = psn.tile([128, 64], F32, name="qps", tag="n")
            nc.tensor.matmul(qps[:], W("Wq3s"), s1s[:], start=True, stop=True)
            qsum = stp.tile([128, 1], F32, name="qsum", tag="hred")
            nc.vector.tensor_reduce(qsum[:], qps[:], mybir.AxisListType.X, ALU.add)
            qsum2 = stp.tile([128, 1], F32, name="qsum2", tag="hv2")
            nc.vector.tensor_scalar_add(qsum2[:], qsum[:], V("s1c"))
            S_h = stp.tile([128, 64], F16, name="S_h0", tag="S_h")
            S_l = stp.tile([128, 64], F16, name="S_l0", tag="S_l")
            nc.vector.tensor_scalar_add(S_h[:], sps[:, 0:64], qsum2[:])
            nc.vector.scalar_tensor_tensor(S_l[:], sps[:, 0:64], qsum2[:], S_h[:],
                                           ALU.add, ALU.subtract)
            nc.tensor.matmul(sps[:, 64:128], W("Wstk"), tys[:], start=True, stop=True)
            se_h = stp.tile([128, 64], F16, name="se_h0", tag="se_h")
            se_l = stp.tile([128, 64], F16, name="se_l0", tag="se_l")
            nc.vector.tensor_scalar_add(se_h[:], sps[:, 64:128], V("bself"))
            nc.vector.scalar_tensor_tensor(se_l[:], sps[:, 64:128], V("bself"), se_h[:],
                                           ALU.add, ALU.subtract)
            dbg.update(S_h0=S_h, S_l0=S_l, se_h0=se_h)

            # ---- t=0 LSTM from G32 (h0 = c0 = 0); gate cols (i,f,g,o)
            G3d = G32[:].rearrange("p (g r) -> p g r", g=4)

            def g_ap(t):
                return G3d[:, :, t * 64:t * 64 + 64]   # [128, 4, 64]

            def lstm_tail(t, Tt, ctil_prev):
                # Tt layout: [0:64]=sigmoid(zf), [64:128]=tanh(zi/2),
                #            [128:192]=tanh(zg), [192:256]=tanh(zo/2)
                ctil = stp.tile([128, 64], F32, name=f"ctil{t}", tag="ctil")
                if ctil_prev is None:
                    nc.vector.scalar_tensor_tensor(ctil[:], Tt[:, 64:128], 1.0,
                                                   Tt[:, 128:192], ALU.add, ALU.mult)
                else:
                    # u = sigmoid(zf) * ctil_prev on GPSIMD (pure TT),
                    # in parallel with v on DVE
                    u = stp.tile([128, 64], F32, name=f"u{t}", tag="u")
                    nc.gpsimd.tensor_mul(u[:], Tt[:, 0:64], ctil_prev[:])
                    v = stp.tile([128, 64], F32, name=f"v{t}", tag="v")
                    nc.vector.scalar_tensor_tensor(v[:], Tt[:, 64:128], 1.0,
                                                   Tt[:, 128:192], ALU.add, ALU.mult)
                    nc.vector.tensor_add(ctil[:], u[:], v[:])
                tc32 = stp.tile([128, 64], F32, name=f"tc{t}", tag="tc")
                nc.scalar.activation(tc32[:], ctil[:], AF.Tanh, scale=0.5)
                h32 = stp.tile([128, 64], F32, name=f"h32_{t}", tag="h32")
                hred = stp.tile([128, 1], F32, name=f"hred{t}", tag="hred")
                nc.vector.scalar_tensor_tensor(h32[:], Tt[:, 192:256], 1.0,
                                               tc32[:], ALU.add, ALU.mult,
                                               accum_out=hred[:])
                hsl = slice(t * 64, t * 64 + 64)
                nc.vector.tensor_copy(hah[:, hsl], h32[:])
                hrh = stp.tile([128, 1], F16, name=f"hrh{t}", tag="hrh")
                nc.vector.tensor_copy(hrh[:], hred[:])
                hrl = None
                if t <= 8:
                    nc.vector.tensor_sub(hal[:, hsl], h32[:], hah[:, hsl])
                    hrl = stp.tile([128, 1], F16, name=f"hrl{t}", tag="hrl")
                    nc.vector.tensor_sub(hrl[:], hred[:], hrh[:])
                return ctil, hrh, hrl

            T0 = stp.tile([128, 256], F32, name="T0", tag="T")
            nc.scalar.activation(T0[:, 64:256], G3d[:, 1:4, 0:64], AF.Tanh)
            ctil, hrh, hrl = lstm_tail(0, T0, None)
            for g in range(4):
                emit_G2(g, 64, 384)     # t=1..5, overlaps the t0 LSTM tail

            def emit_pred(c0, c1):
                s = slice(c0, c1)
                pps = psG.tile([128, c1 - c0], F32, name=f"pps{c0}", tag="G")
                nc.tensor.matmul(pps[:], W("Wpredhh"), hah[:, s], start=True, stop=False)
                nc.tensor.matmul(pps[:], W("Wpredhl"), hah[:, s], start=False, stop=False)
                nc.tensor.matmul(pps[:], W("Wpredhh"), hal[:, s], start=False, stop=True)
                nc.scalar.activation(out32[:, s], pps[:], AF.Relu, bias=V("b_pred"))
                nc.sync.dma_start(out_ap[:, s], out32[:, s])

            # ---- recurrence steps t=1..11
            for t in range(1, T):
                hp = slice((t - 1) * 64, t * 64)

                # ne psum [128,64]: S/se terms first (ready early), then h,
                # then the hred broadcast terms (via hrep from GPSIMD)
                full = t <= 8   # steps 9-11: 2-term fp16 (error amplified <= 4x)
                nps = psn.tile([128, 65], F32, name=f"nps{t}", tag="n")
                nc.tensor.matmul(nps[:, 0:64], W("W3eh"), S_h[:], start=True, stop=False)
                nc.tensor.matmul(nps[:, 0:64], W("W3el"), S_h[:], start=False, stop=False)
                if full:
                    nc.tensor.matmul(nps[:, 0:64], W("W3eh"), S_l[:], start=False, stop=False)
                nc.tensor.matmul(nps[:, 0:64], W("Wsseh"), se_h[:], start=False, stop=False)
                nc.tensor.matmul(nps[:, 0:64], W("Wssel"), se_h[:], start=False, stop=False)
                if full:
                    nc.tensor.matmul(nps[:, 0:64], W("Wsseh"), se_l[:], start=False, stop=False)
                nc.tensor.matmul(nps[:, 0:64], W("Wcohh"), hah[:, hp], start=False, stop=False)
                nc.tensor.matmul(nps[:, 0:64], W("Wcohl"), hah[:, hp], start=False,
                                 stop=not full)
                if full:
                    nc.tensor.matmul(nps[:, 0:64], W("Wcohh"), hal[:, hp], start=False, stop=True)
                nc.tensor.matmul(nps[:, 64:65], W("W2ehh"), hrh[:], start=True, stop=False)
                nc.tensor.matmul(nps[:, 64:65], W("W2ehl"), hrh[:], start=False,
                                 stop=not full)
                if full:
                    nc.tensor.matmul(nps[:, 64:65], W("W2ehh"), hrl[:], start=False, stop=True)
                ne_h = stp.tile([128, 64], F16, name=f"ne_h{t}", tag="ne_h")
                nc.vector.tensor_scalar_add(ne_h[:], nps[:, 0:64], nps[:, 64:65])
                if full:
                    ne_l = stp.tile([128, 64], F16, name=f"ne_l{t}", tag="ne_l")
                    nc.vector.scalar_tensor_tensor(ne_l[:], nps[:, 0:64], nps[:, 64:65],
                                                   ne_h[:], ALU.add, ALU.subtract)

                # gates psum [128, 256] (i,f,g,o): h-chunks of ALL gates first
                gps = psg.tile([128, 256], F32, name=f"gps{t}", tag="g")
                nc.tensor.matmul(gps[:], i32[:], g_ap(t), start=True, stop=False,
                                 skip_group_check=True)
                for g in range(4):
                    gsl = gps[:, g * 64:(g + 1) * 64]
                    nc.tensor.matmul(gsl, Wg("Whh2h", g), hah[:, hp], start=False,
                                     stop=False, skip_group_check=True)
                    nc.tensor.matmul(gsl, Wg("Whh2l", g), hah[:, hp], start=False,
                                     stop=False, skip_group_check=True)
                    if full:
                        nc.tensor.matmul(gsl, Wg("Whh2h", g), hal[:, hp], start=False,
                                         stop=False, skip_group_check=True)
                    nc.tensor.matmul(gsl, Wg("Wihneh", g), ne_h[:], start=False,
                                     stop=False, skip_group_check=True)
                    nc.tensor.matmul(gsl, Wg("Wihnel", g), ne_h[:], start=False,
                                     stop=(g == 3 and not full), skip_group_check=True)
                    if full:
                        nc.tensor.matmul(gsl, Wg("Wihneh", g), ne_l[:], start=False,
                                         stop=(g == 3), skip_group_check=True)

                Tt = stp.tile([128, 256], F32, name=f"T{t}", tag="T")
                nc.scalar.activation(Tt[:, 0:64], gps[:, 0:64], AF.Sigmoid,
                                     scale=2.0)
                nc.scalar.activation(Tt[:, 64:256], gps[:, 64:256], AF.Tanh)

                # carries for t+1 (skip at last step)
                if t < T - 1:
                    sps2 = pss.tile([128, 130], F32, name=f"sps{t}", tag="s")
                    nc.tensor.matmul(sps2[:, 0:64], W("W3h"), S_h[:], start=True, stop=False)
                    nc.tensor.matmul(sps2[:, 0:64], W("W3l"), S_h[:], start=False, stop=False)
                    if full:
                        nc.tensor.matmul(sps2[:, 0:64], W("W3h"), S_l[:], start=False, stop=False)
                    nc.tensor.matmul(sps2[:, 0:64], W("AW1hh"), hah[:, hp], start=False, stop=False)
                    nc.tensor.matmul(sps2[:, 0:64], W("AW1hl"), hah[:, hp], start=False,
                                     stop=not full)
                    if full:
                        nc.tensor.matmul(sps2[:, 0:64], W("AW1hh"), hal[:, hp], start=False, stop=True)
                    nc.tensor.matmul(sps2[:, 128:129], W("W2hh"), hrh[:], start=True, stop=False)
                    nc.tensor.matmul(sps2[:, 128:129], W("W2hl"), hrh[:], start=False,
                                     stop=not full)
                    if full:
                        nc.tensor.matmul(sps2[:, 128:129], W("W2hh"), hrl[:], start=False, stop=True)
                    nc.tensor.matmul(sps2[:, 64:128], W("Wssh"), se_h[:], start=True, stop=False)
                    nc.tensor.matmul(sps2[:, 64:128], W("Wssl"), se_h[:], start=False, stop=False)
                    if full:
                        nc.tensor.matmul(sps2[:, 64:128], W("Wssh"), se_l[:], start=False, stop=False)
                    nc.tensor.matmul(sps2[:, 64:128], W("Wsh2h"), hah[:, hp], start=False, stop=False)
                    nc.tensor.matmul(sps2[:, 64:128], W("Wsh2l"), hah[:, hp], start=False, stop=False)
                    if full:
                        nc.tensor.matmul(sps2[:, 64:128], W("Wsh2h"), hal[:, hp], start=False, stop=False)
                    nc.tensor.matmul(sps2[:, 64:128], W("Wstk"), tys[:], start=False, stop=True)
                    hv2 = stp.tile([128, 1], F32, name=f"hv2{t}", tag="hv2")
                    nc.scalar.activation(hv2[:], sps2[:, 128:129], AF.Identity,
                                         bias=V("sbias"))
                    S_h = stp.tile([128, 64], F16, name=f"S_h{t}", tag="S_h")
                    nc.scalar.activation(S_h[:], sps2[:, 0:64], AF.Identity,
                                         bias=hv2[:])
                    se_h = stp.tile([128, 64], F16, name=f"se_h{t}", tag="se_h")
                    nc.scalar.activation(se_h[:], sps2[:, 64:128], AF.Identity,
                                         bias=V("bself"))
                    if t <= 7:
                        S_l = stp.tile([128, 64], F16, name=f"S_l{t}", tag="S_l")
                        nc.vector.scalar_tensor_tensor(S_l[:], sps2[:, 0:64], hv2[:],
                                                       S_h[:], ALU.add, ALU.subtract)
                        se_l = stp.tile([128, 64], F16, name=f"se_l{t}", tag="se_l")
                        nc.vector.scalar_tensor_tensor(se_l[:], sps2[:, 64:128],
                                                       V("bself"), se_h[:],
                                                       ALU.add, ALU.subtract)

                ctil, hrh, hrl = lstm_tail(t, Tt, ctil)
                if 1 <= t <= 4:
                    emit_G2(t - 1, 384, 576)    # t=6..8 slices
                elif 5 <= t <= 8:
                    emit_G2(t - 5, 576, 768)    # t=9..11 slices
                if t == 6:
                    emit_pred(0, 384)
                elif t == 10:
                    emit_pred(384, 704)

            emit_pred(704, 768)

    _split_excess_waits(nc)
    return dbg


# ---------------------------------------------------------------------------
def _make_in_maps(inputs):
    Pe, Pl, vec32 = _prep_weights(inputs)
    we = Pe.blob()
    wlb = Pl.blob()
    in_maps = []
    for b in range(B):
        ci = _prep_core_inputs(inputs, b)
        ci["wearly"] = we
        ci["wlate"] = wlb
        ci["vec32"] = vec32
        ci["ident32"] = np.eye(128, dtype=np.float32)
        in_maps.append(ci)
    ones_fast = all(m.pop("ones_fast") for m in in_maps)
    if not ones_fast:
        # all cores share one SPMD program; rebuild blobs on the general layout
        for m_ in in_maps:
            if m_["inblob"].shape[1] != 5568:
                big = np.zeros((128, 5568), np.float16)
                big[:, 0:3264] = m_["inblob"]
                m_["inblob"] = big
    return Pe, Pl, in_maps, ones_fast


def kernel(**inputs):
    Pe, Pl, in_maps, ones_fast = _make_in_maps(inputs)
    nc = bass.Bass("TRN2", target_bir_lowering=False, debug=False, num_devices=B)
    _build(nc, Pe.items, Pe.cols, Pl.items, Pl.cols, ones_fast)
    res = run_bass_kernel_spmd(nc, in_maps, list(range(B)))
    outs = []
    for b in range(B):
        o = res.results[b]["out"]                    # [128, 768] (O, t*64+a)
        outs.append(o.T.reshape(T, A, O).transpose(1, 0, 2))
    return np.stack(outs).astype(np.float32)         # [B, A, T, O]


if __name__ == "__main__":
    d = np.load("/root/problem/expected.npz")
    inputs = {k: d[k] for k in d.files if k != "expected"}
    out = kernel(**inputs)
    exp = d["expected"]
    err = np.abs(out - exp).max()
    print("absmax err:", err, "rel:", err / np.abs(exp).max())

